# revision 4
# baseline (speedup 1.0000x reference)
"""CFN cell on 8 TRN2 NeuronCores — tensor-parallel over H, bf16 matmuls.

v3: decoupled s-phase / x-phase PSUM pipelines.

  - s-phase (state @ [Wtu|Weu]) accumulates into its own PSUM bank and is
    immediately drained (+bias) to an SBUF `pre_s` tile by a vector STT.
    This frees the bank after ~3.4 us, so s-phases can run arbitrarily far
    ahead of x-phases (pipe=12) without exhausting the 8 PSUM banks.
  - The deep s-ahead prologue (12 s-phases = ~41 us of PE work that needs
    only wte + stw windows) hides the entire x-operand preload
    (wtwe/wwx/xtw) behind real matmul work.
  - Startup DMAs are issued in consumption order, round-robin across 4
    engine queues (sync/scalar/vector/gpsimd), so the first matmul starts
    after ~200 KB instead of ~3 MB.
  - tanh(state) tiles are precomputed per window on the scalar engine as
    soon as sbh lands; epilogue tiles are bf16 (2x DVE rate), output is
    bf16 (host upcasts).
"""

import numpy as np
import ml_dtypes
from contextlib import ExitStack

import concourse.bass as bass
import concourse.mybir as mybir
import concourse.tile as tile
from concourse import bacc
from concourse.bass_utils import run_bass_kernel_spmd

F32 = mybir.dt.float32
BF16 = mybir.dt.bfloat16
AF = mybir.ActivationFunctionType
ALU = mybir.AluOpType

B, D_IN, H, NCORES = 4096, 2048, 2048, 8
H_LOC = H // NCORES  # 256
WIN = 256

TRACE = False
LAST_RESULTS = None
_NC_CACHE = {}


def build(nc, b, d_in, d_state, h_loc, pipe=12):
    n_bt = b // 128
    ktx, kts = d_in // 128, d_state // 128
    h2 = 2 * h_loc
    tpw = WIN // 128
    n_win = b // WIN

    xt = nc.dram_tensor("xt", [n_win, 128, ktx, WIN], BF16,
                        kind="ExternalInput").ap()
    st = nc.dram_tensor("st", [n_win, 128, kts, WIN], BF16,
                        kind="ExternalInput").ap()
    sbh = nc.dram_tensor("sbh", [n_win, 128, tpw, h_loc], BF16,
                         kind="ExternalInput").ap()
    wte = nc.dram_tensor("wte", [128, kts, h2], BF16, kind="ExternalInput").ap()
    wtwe = nc.dram_tensor("wtwe", [128, ktx, h2], BF16,
                          kind="ExternalInput").ap()
    wwx = nc.dram_tensor("wwx", [128, ktx, h_loc], BF16,
                         kind="ExternalInput").ap()
    bias = nc.dram_tensor("bias", [h2], F32, kind="ExternalInput").ap()
    out = nc.dram_tensor("h_out", [b, h_loc], BF16, kind="ExternalOutput").ap()

    with tile.TileContext(nc) as tc, ExitStack() as ctx:
        consts = ctx.enter_context(tc.tile_pool(name="consts", bufs=1))
        stwp = ctx.enter_context(tc.tile_pool(name="stwp", bufs=4))
        xtwp = ctx.enter_context(tc.tile_pool(name="xtwp", bufs=4))
        sbhp = ctx.enter_context(tc.tile_pool(name="sbhp", bufs=4))
        tshp = ctx.enter_context(tc.tile_pool(name="tshp", bufs=8))
        presp = ctx.enter_context(tc.tile_pool(name="presp", bufs=pipe + 2))
        temps = ctx.enter_context(tc.tile_pool(name="temps", bufs=3))
        psum = ctx.enter_context(tc.tile_pool(name="psum", bufs=1, space="PSUM"))

        wte_sb = consts.tile([128, kts, h2], BF16, tag="wte")
        wtwe_sb = consts.tile([128, ktx, h2], BF16, tag="wtwe")
        wwx_sb = consts.tile([128, ktx, h_loc], BF16, tag="wwx")
        bias_bc = consts.tile([128, h2], F32, tag="bias_bc")

        stw_map, xtw_map, tsh_map = {}, {}, {}

        # round-robin DMA queue dispatcher for the prologue
        # (only SP/Activation hwdge + gpsimd swdge can initiate DMAs)
        queues = [nc.sync, nc.scalar, nc.gpsimd]
        qi = [0]

        def dq(out_, in_):
            queues[qi[0] % len(queues)].dma_start(out=out_, in_=in_)
            qi[0] += 1

        def load_stw(iw, eng=None, nq=4):
            stw = stwp.tile([128, kts, WIN], BF16, tag="stw", name=f"stw{iw}")
            step = kts // nq
            for c in range(0, kts, step):
                ce = c + step
                if eng is None:
                    dq(stw[:, c:ce, :], st[iw, :, c:ce, :])
                else:
                    eng.dma_start(out=stw[:, c:ce, :], in_=st[iw, :, c:ce, :])
            stw_map[iw] = stw

        def load_xtw(iw, eng=None, nq=4):
            xtw = xtwp.tile([128, ktx, WIN], BF16, tag="xtw", name=f"xtw{iw}")
            step = ktx // nq
            for c in range(0, ktx, step):
                ce = c + step
                if eng is None:
                    dq(xtw[:, c:ce, :], xt[iw, :, c:ce, :])
                else:
                    eng.dma_start(out=xtw[:, c:ce, :], in_=xt[iw, :, c:ce, :])
            xtw_map[iw] = xtw

        def load_sbh(iw, eng=None):
            sbh_t = sbhp.tile([128, tpw, h_loc], BF16, tag="sbh",
                              name=f"sbh{iw}")
            (eng or nc.gpsimd).dma_start(out=sbh_t, in_=sbh[iw])
            return sbh_t

        def make_tsh(iw, eng=None):
            sbh_t = load_sbh(iw, eng=eng)
            tsh = tshp.tile([128, tpw, h_loc], BF16, tag="tsh",
                            name=f"tsh{iw}")
            nc.scalar.activation(tsh, sbh_t, AF.Tanh)
            tsh_map[iw] = tsh

        # ── Prologue DMA choreography ──────────────────────────────────
        # 1. stw0 + wte chunk-interleaved (s-phase 0 consumption order).
        stw0 = stwp.tile([128, kts, WIN], BF16, tag="stw", name="stw0")
        stw_map[0] = stw0
        for k in range(kts):
            dq(stw0[:, k, :], st[0, :, k, :])
            dq(wte_sb[:, k, :], wte[:, k, :])
        # 2. next stw windows (consumed at 3.4 us per window).
        load_stw(1)
        load_stw(2)
        # 3. sbh/bias (tanh(state) precompute feeds the epilogue much later).
        bias_bcast_ap = bass.AP(
            tensor=bias.tensor, offset=bias.offset,
            ap=[[0, 128]] + list(bias.ap),
        )
        nc.gpsimd.dma_start(out=bias_bc, in_=bias_bcast_ap)
        load_stw(3)
        # 4. x-phase operands: needed only after `pipe` s-phases (~40 us).
        for c in range(0, ktx, 2):
            dq(wtwe_sb[:, c:c + 2, :], wtwe[:, c:c + 2, :])
        for c in range(0, ktx, 4):
            dq(wwx_sb[:, c:c + 4, :], wwx[:, c:c + 4, :])
        load_stw(4)
        load_xtw(0)
        load_xtw(1)
        load_stw(5)
        load_xtw(2)

        def s_phase(ib):
            iw = ib // tpw
            if iw + 3 not in stw_map and iw + 3 < n_win:
                load_stw(iw + 3, eng=nc.sync)
            if ib % tpw == 0:
                make_tsh(iw)
            for key in [k2 for k2 in stw_map if k2 < iw - 1]:
                del stw_map[key]
            stw = stw_map[iw]
            bcol = (ib % tpw) * 128
            ps_s = psum.tile([128, h2], F32, tag="ps_s", bufs=3,
                             name=f"ps_s{ib}")
            for k in range(kts):
                nc.tensor.matmul(
                    ps_s, stw[:, k, bcol:bcol + 128], wte_sb[:, k, :],
                    start=(k == 0), stop=(k == kts - 1),
                )
            pre_s = presp.tile([128, h2], F32, tag="pre_s", name=f"pre_s{ib}")
            nc.vector.scalar_tensor_tensor(
                pre_s, ps_s, 1.0, bias_bc, op0=ALU.mult, op1=ALU.add,
            )
            return pre_s

        def x_phase(ib, pre_s):
            iw = ib // tpw
            if iw + 3 not in xtw_map and iw + 3 < n_win:
                load_xtw(iw + 3, eng=nc.sync)
            for m in (xtw_map, tsh_map):
                for key in [k2 for k2 in m if k2 < iw - 1]:
                    del m[key]
            xtw = xtw_map[iw]
            tsh = tsh_map[iw]
            it = ib % tpw
            bcol = it * 128
            ps_x = psum.tile([128, h2], F32, tag="ps_x", bufs=2,
                             name=f"ps_x{ib}")
            ps_w = psum.tile([128, h_loc], F32, tag="ps_w", bufs=2,
                             name=f"ps_w{ib}")
            for k in range(ktx):
                nc.tensor.matmul(
                    ps_x, xtw[:, k, bcol:bcol + 128], wtwe_sb[:, k, :],
                    start=(k == 0), stop=(k == ktx - 1),
                )
                nc.tensor.matmul(
                    ps_w, xtw[:, k, bcol:bcol + 128], wwx_sb[:, k, :],
                    start=(k == 0), stop=(k == ktx - 1),
                )

            pre = temps.tile([128, h2], F32, tag="pre", name=f"pre{ib}")
            nc.vector.scalar_tensor_tensor(
                pre, ps_x, 1.0, pre_s, op0=ALU.mult, op1=ALU.add,
            )
            theta = temps.tile([128, h_loc], BF16, tag="theta", name=f"th{ib}")
            nc.scalar.activation(theta, pre[:, :h_loc], AF.Sigmoid)
            eta = temps.tile([128, h_loc], BF16, tag="eta", name=f"et{ib}")
            nc.scalar.activation(eta, pre[:, h_loc:], AF.Sigmoid)
            twx = temps.tile([128, h_loc], BF16, tag="twx", name=f"twx{ib}")
            nc.scalar.activation(twx, ps_w, AF.Tanh)

            p1 = temps.tile([128, h_loc], BF16, tag="p1", name=f"p1{ib}")
            nc.vector.tensor_mul(p1, theta, tsh[:, it, :])
            p2 = temps.tile([128, h_loc], BF16, tag="p2", name=f"p2{ib}")
            nc.vector.tensor_mul(p2, eta, twx)
            ho = temps.tile([128, h_loc], BF16, tag="ho", name=f"ho{ib}")
            nc.vector.tensor_add(ho, p1, p2)
            bsl = slice(ib * 128, (ib + 1) * 128)
            nc.gpsimd.dma_start(out=out[bsl, :], in_=ho)

        pending = [(ib, s_phase(ib)) for ib in range(min(pipe, n_bt))]
        for ib in range(pipe, n_bt):
            pib, ps = pending.pop(0)
            x_phase(pib, ps)
            pending.append((ib, s_phase(ib)))
        for pib, ps in pending:
            x_phase(pib, ps)

    nc.compile()
    return nc


def _get_nc():
    key = (B, D_IN, H, H_LOC)
    if key not in _NC_CACHE:
        nc = bacc.Bacc("TRN2", target_bir_lowering=False, debug=False,
                       num_devices=NCORES)
        _NC_CACHE[key] = build(nc, B, D_IN, H, H_LOC)
    return _NC_CACHE[key]


BF = ml_dtypes.bfloat16


def _pack_acts(at):  # at: [D, B] (transposed activations), bf16
    d, b_ = at.shape
    kt, n_win = d // 128, b_ // WIN
    # (t*128+p, iw*WIN+j) -> [iw, p, t, j]
    return np.ascontiguousarray(
        at.reshape(kt, 128, n_win, WIN).transpose(2, 1, 0, 3)
    )


def _pack_w(wm):  # wm: [D, h] -> [p, t, h], bf16
    d, h = wm.shape
    kt = d // 128
    return np.ascontiguousarray(wm.reshape(kt, 128, h).transpose(1, 0, 2))


def make_in_maps(inputs):
    x = np.asarray(inputs["inputs"], dtype=np.float32)
    s = np.asarray(inputs["state"], dtype=np.float32)
    xb = x.astype(BF)
    sb = s.astype(BF)
    w = {
        k: np.asarray(inputs[k], dtype=np.float32).astype(BF)
        for k in ("theta_u_w", "theta_w_w", "eta_u_w", "eta_w_w", "wx_w")
    }
    bt_full = np.asarray(inputs["theta_w_b"], dtype=np.float32)
    be_full = np.asarray(inputs["eta_w_b"], dtype=np.float32)

    xt_p = _pack_acts(np.ascontiguousarray(xb.T))  # shared by all cores
    st_p = _pack_acts(np.ascontiguousarray(sb.T))
    n_win, tpw = B // WIN, WIN // 128

    in_maps = []
    for c in range(NCORES):
        hsl = slice(c * H_LOC, (c + 1) * H_LOC)
        sbh_c = np.ascontiguousarray(
            sb[:, hsl].reshape(n_win, tpw, 128, H_LOC).transpose(0, 2, 1, 3)
        )
        in_maps.append({
            "xt": xt_p,
            "st": st_p,
            "sbh": sbh_c,
            "wte": _pack_w(np.concatenate(
                [w["theta_u_w"][:, hsl], w["eta_u_w"][:, hsl]], axis=1)),
            "wtwe": _pack_w(np.concatenate(
                [w["theta_w_w"][:, hsl], w["eta_w_w"][:, hsl]], axis=1)),
            "wwx": _pack_w(w["wx_w"][:, hsl]),
            "bias": np.ascontiguousarray(
                np.concatenate([bt_full[hsl], be_full[hsl]])
            ),
        })
    return in_maps


def kernel(**inputs):
    global LAST_RESULTS
    in_maps = make_in_maps(inputs)
    nc = _get_nc()
    res = run_bass_kernel_spmd(nc, in_maps, core_ids=list(range(NCORES)),
                               trace=TRACE)
    LAST_RESULTS = res

    h = np.empty((B, H), np.float32)
    for c in range(NCORES):
        h[:, c * H_LOC:(c + 1) * H_LOC] = \
            res.results[c]["h_out"].astype(np.float32)
    return (h, h)
